# revision 13
# baseline (speedup 1.0000x reference)
"""Trainium2 Bass kernel for nn_AssocModel (gnn_message_passing).

Computes, for each (tau, track, future-node) pair, a 3-layer edge-MLP logit:
    logits[n,m,t] = W3^T relu(W2^T relu(U[n,m] + V[n,t]) + b2) + b3
where the pre-ReLU first layer decomposes exactly into an outer sum of a
per-(tau,track) vector U and a per-(tau,future) vector V:
    U[n,m] = Wi^T hi[m] - xpred[n,m,0]*Wx - xpred[n,m,1]*Wy
    V[n,t] = Wj^T hj[n,t] + xj[0]*Wx + xj[1]*Wy + tau_norm[n]*Wt + b1
|W3| is folded into W2's columns (and b2), leaving a +/-1 sign vector as the
final contraction, which runs as a column-packed PE matmul.

Sharding: data-parallel over the 1024 tracks -> 128 per NeuronCore, weights
and fut_nodes replicated; no collectives. Output gathered host-side.
"""
import sys

import numpy as np

try:
    import concourse.bass as bass
except ImportError:  # staged repo location inside the container
    sys.path.insert(0, "/opt/trn_rl_repo")
    import concourse.bass as bass

import concourse.bacc as bacc
import concourse.mybir as mybir
import concourse.tile as tile
from concourse.bass_utils import run_bass_kernel_spmd
from concourse.masks import make_identity

F32 = mybir.dt.float32
BF16 = mybir.dt.bfloat16
ALU = mybir.AluOpType
ACTF = mybir.ActivationFunctionType

N_CORES = 8
NM, K_HIST, IN_DIM = 1024, 10, 16
N_TAU, NT = 2, 512
H = 128      # node encoder out
EH = 128     # edge hidden
M_LOC = NM // N_CORES  # 128 tracks per core

# tuning knobs
DVE_H2_EVERY = 6   # every k-th h2-relu pair goes to DVE instead of ACT
H1_BUFS = 6
HP_BUFS = 3        # [128,1024] f32 psum pairs in flight (2 banks each)
H2_BUFS = 3


def build_kernel():
    nc = bacc.Bacc(name="assoc_edge_mlp")

    def inp(name, shape):
        return nc.declare_dram_parameter(name, list(shape), F32, isOutput=False)

    curr = inp("curr_nodes", [M_LOC, IN_DIM])
    hist = inp("hist_xy", [M_LOC, K_HIST, 2])
    mask = inp("hist_mask", [M_LOC, K_HIST])
    fut = inp("fut_nodes", [N_TAU * NT, IN_DIM])
    ec_W1 = inp("ec_W1", [IN_DIM, H]); ec_b1 = inp("ec_b1", [1, H])
    ec_W2 = inp("ec_W2", [H, H]);      ec_b2 = inp("ec_b2", [1, H])
    ef_W1 = inp("ef_W1", [IN_DIM, H]); ef_b1 = inp("ef_b1", [1, H])
    ef_W2 = inp("ef_W2", [H, H]);      ef_b2 = inp("ef_b2", [1, H])
    e_W1 = inp("e_W1", [2 * H + 3, EH])
    e_b1 = inp("e_b1", [1, EH])
    e_W2 = inp("e_W2", [EH, EH])
    e_b2 = inp("e_b2", [1, EH])
    e_W3 = inp("e_W3", [1, EH])  # flattened [EH,1] -> row
    e_b3 = inp("e_b3", [1, 1])
    dummy = inp("dummy_bias", [1, 1])
    out = nc.declare_dram_parameter("out", [N_TAU, M_LOC, NT + 1], F32, isOutput=True)

    from contextlib import ExitStack
    with tile.TileContext(nc) as tc, ExitStack() as stack:
        consts = stack.enter_context(tc.tile_pool(name="consts", bufs=1))

        # ---------------- static / DMA'd constants ----------------
        identity = consts.tile([128, 128], F32)
        make_identity(nc, identity)
        ones_row = consts.tile([1, NT], F32)
        nc.vector.memset(ones_row, 1.0)

        curr_pm = consts.tile([M_LOC, IN_DIM], F32)
        nc.sync.dma_start(out=curr_pm, in_=curr.ap())
        hist_pm = consts.tile([M_LOC, K_HIST, 2], F32)
        nc.sync.dma_start(out=hist_pm, in_=hist.ap())
        mask_pm = consts.tile([M_LOC, K_HIST], F32)
        nc.sync.dma_start(out=mask_pm, in_=mask.ap())
        fut_pm = consts.tile([128, 8, IN_DIM], F32)  # 1024 rows folded 8x
        nc.sync.dma_start(
            out=fut_pm,
            in_=fut.ap().rearrange("(k p) c -> p k c", p=128),
        )

        ecW1_sb = consts.tile([IN_DIM, H], F32)
        nc.sync.dma_start(out=ecW1_sb, in_=ec_W1.ap())
        ecW2_sb = consts.tile([H, H], F32)
        nc.sync.dma_start(out=ecW2_sb, in_=ec_W2.ap())
        efW1_sb = consts.tile([IN_DIM, H], F32)
        nc.sync.dma_start(out=efW1_sb, in_=ef_W1.ap())
        efW2_sb = consts.tile([H, H], F32)
        nc.sync.dma_start(out=efW2_sb, in_=ef_W2.ap())

        Wi_sb = consts.tile([H, EH], F32)
        nc.sync.dma_start(out=Wi_sb, in_=e_W1.ap()[0:H, :])
        Wj_sb = consts.tile([H, EH], F32)
        nc.sync.dma_start(out=Wj_sb, in_=e_W1.ap()[H:2 * H, :])
        Wxy_sb = consts.tile([2, EH], F32)
        nc.sync.dma_start(out=Wxy_sb, in_=e_W1.ap()[2 * H:2 * H + 2, :])
        Wt_row = consts.tile([1, EH], F32)
        nc.sync.dma_start(out=Wt_row, in_=e_W1.ap()[2 * H + 2:2 * H + 3, :])
        b1_row = consts.tile([1, EH], F32)
        nc.sync.dma_start(out=b1_row, in_=e_b1.ap())
        eW2_sb = consts.tile([EH, EH], F32)
        nc.sync.dma_start(out=eW2_sb, in_=e_W2.ap())
        w3_row = consts.tile([1, EH], F32)
        nc.sync.dma_start(out=w3_row, in_=e_W3.ap())

        # small per-channel vectors, loaded as rows then transposed to columns
        rows_sb = consts.tile([8, 128], F32)
        nc.vector.memset(rows_sb, 0.0)
        nc.sync.dma_start(out=rows_sb[0:1, :], in_=ec_b1.ap())
        nc.sync.dma_start(out=rows_sb[1:2, :], in_=ec_b2.ap())
        nc.sync.dma_start(out=rows_sb[2:3, :], in_=ef_b1.ap())
        nc.sync.dma_start(out=rows_sb[3:4, :], in_=ef_b2.ap())
        nc.sync.dma_start(out=rows_sb[4:5, :], in_=e_b2.ap())
        nc.sync.dma_start(out=rows_sb[5:6, :], in_=e_W3.ap())
        b3_col = consts.tile([128, 1], F32)
        nc.sync.dma_start(out=b3_col, in_=e_b3.ap().broadcast_to([128, 1]))

        with tc.tile_pool(name="prep_ps", bufs=2, space="PSUM") as pps, \
             tc.tile_pool(name="prep_sb", bufs=1) as psb:
            # transpose the small rows -> columns [128, 8]
            colT = pps.tile([128, 8], F32, tag="pp")
            nc.tensor.transpose(colT, rows_sb, identity[0:8, 0:8])
            cols_sb = consts.tile([128, 8], F32)
            nc.scalar.activation(cols_sb, colT, ACTF.Copy)
            ecb1_c = cols_sb[:, 0:1]; ecb2_c = cols_sb[:, 1:2]
            efb1_c = cols_sb[:, 2:3]; efb2_c = cols_sb[:, 3:4]
            eb2_c = cols_sb[:, 4:5]; w3_c = cols_sb[:, 5:6]

            # ---- derived scorer weights: fold |w3| into W2 cols, sign vec ----
            sg = psb.tile([128, 1], F32)
            nc.scalar.activation(sg, w3_c, ACTF.Sign)
            sg2 = psb.tile([128, 1], F32)
            nc.vector.tensor_mul(sg2, sg, sg)
            s_col = consts.tile([128, 1], F32)
            nc.vector.tensor_sub(s_col, sg, sg2)
            nc.vector.tensor_scalar_add(s_col, s_col, 1.0)  # zeros -> +1
            s_bf = consts.tile([128, 1], BF16)
            nc.vector.tensor_copy(s_bf, s_col)
            # S_buf: 32 packed [128,32] lhsT matrices, S_j has s in column j
            # (stride-32 packing puts column j of matrix j at flat col 33*j)
            S_buf = consts.tile([128, 32 * 32], BF16)
            nc.vector.memset(S_buf, 0.0)
            S_diag = S_buf[:, 0:1024:33]
            nc.vector.tensor_copy(S_diag, s_bf.broadcast_to([128, 32]))
            w3a_c = consts.tile([128, 1], F32)
            nc.vector.tensor_mul(w3a_c, w3_c, s_col)        # |w3| column
            b2p_c = consts.tile([128, 1], F32)
            nc.vector.tensor_mul(b2p_c, eb2_c, w3a_c)

            w3a_bc = consts.tile([128, EH], F32)
            nc.sync.dma_start(out=w3a_bc, in_=e_W3.ap().broadcast_to([128, EH]))
            nc.scalar.activation(w3a_bc, w3a_bc, ACTF.Abs)
            W2p_bf = consts.tile([EH, EH], BF16)
            nc.vector.tensor_mul(W2p_bf, eW2_sb, w3a_bc)

            # cvec_n = b1 + tau_norm[n] * Wt   (rows [1, EH])
            cv = []
            for n in range(N_TAU):
                cvn = consts.tile([1, EH], F32, name=f"cv{n}")
                nc.vector.tensor_scalar(
                    out=cvn, in0=Wt_row, scalar1=float((n + 1) / N_TAU),
                    scalar2=None, op0=ALU.mult)
                nc.vector.tensor_add(cvn, cvn, b1_row)
                cv.append(cvn)

            # ---------------- velocity regression (partition-major) ----------
            t_bc = psb.tile([M_LOC, K_HIST], F32)
            nc.gpsimd.iota(t_bc, pattern=[[1, K_HIST]], base=0,
                           channel_multiplier=0,
                           allow_small_or_imprecise_dtypes=True)
            nc.vector.tensor_scalar_add(t_bc, t_bc, float(-(K_HIST - 1)))
            hx = hist_pm[:, :, 0]
            hy = hist_pm[:, :, 1]

            S0 = psb.tile([M_LOC, 1], F32)
            nc.vector.reduce_sum(S0, mask_pm, axis=mybir.AxisListType.X)
            r0 = psb.tile([M_LOC, 1], F32)
            nc.vector.tensor_scalar_max(S0, S0, 1.0)
            nc.vector.reciprocal(r0, S0)

            tmp_k = psb.tile([M_LOC, K_HIST], F32)
            t_mean = psb.tile([M_LOC, 1], F32)
            nc.vector.tensor_mul(tmp_k, mask_pm, t_bc)
            nc.vector.reduce_sum(t_mean, tmp_k, axis=mybir.AxisListType.X)
            nc.vector.tensor_mul(t_mean, t_mean, r0)

            ymx = psb.tile([M_LOC, 1], F32)
            nc.vector.tensor_mul(tmp_k, mask_pm, hx)
            nc.vector.reduce_sum(ymx, tmp_k, axis=mybir.AxisListType.X)
            nc.vector.tensor_mul(ymx, ymx, r0)
            ymy = psb.tile([M_LOC, 1], F32)
            nc.vector.tensor_mul(tmp_k, mask_pm, hy)
            nc.vector.reduce_sum(ymy, tmp_k, axis=mybir.AxisListType.X)
            nc.vector.tensor_mul(ymy, ymy, r0)

            t_c = psb.tile([M_LOC, K_HIST], F32)
            nc.vector.tensor_scalar(out=t_c, in0=t_bc, scalar1=t_mean,
                                    scalar2=None, op0=ALU.subtract)
            nc.vector.tensor_mul(t_c, t_c, mask_pm)
            y_c = psb.tile([M_LOC, K_HIST], F32)

            den = psb.tile([M_LOC, 1], F32)
            nc.vector.tensor_mul(tmp_k, t_c, t_c)
            nc.vector.reduce_sum(den, tmp_k, axis=mybir.AxisListType.X)
            nc.vector.tensor_scalar_max(den, den, 1e-8)
            rden = psb.tile([M_LOC, 1], F32)
            nc.vector.reciprocal(rden, den)

            v2 = psb.tile([M_LOC, 2], F32)
            for ci, (hc, ymc) in enumerate(((hx, ymx), (hy, ymy))):
                nc.vector.tensor_scalar(out=y_c, in0=hc, scalar1=ymc,
                                        scalar2=None, op0=ALU.subtract)
                nc.vector.tensor_mul(y_c, y_c, mask_pm)
                nc.vector.tensor_mul(tmp_k, t_c, y_c)
                num = psb.tile([M_LOC, 1], F32, name=f"num{ci}")
                nc.vector.reduce_sum(num, tmp_k, axis=mybir.AxisListType.X)
                nc.vector.tensor_mul(v2[:, ci:ci + 1], num, rden)

            # P_cols[:, 2n:2n+2] = -(xi + tau_n * v)
            P_cols = psb.tile([M_LOC, 4], F32)
            for n in range(N_TAU):
                sl = P_cols[:, 2 * n:2 * n + 2]
                nc.vector.tensor_scalar(out=sl, in0=v2,
                                        scalar1=float(-(n + 1)),
                                        scalar2=None, op0=ALU.mult)
                nc.vector.tensor_sub(sl, sl, curr_pm[:, 0:2])
            P_rows = []
            for n in range(N_TAU):
                PT_ps = pps.tile([2, M_LOC], F32, tag="pp", name=f"PT{n}")
                nc.tensor.transpose(PT_ps, P_cols[:, 2 * n:2 * n + 2], identity)
                Pr = consts.tile([2, M_LOC], F32, name=f"Pr{n}")
                nc.scalar.activation(Pr, PT_ps, ACTF.Copy)
                P_rows.append(Pr)

            # ---------------- input transposes ----------------
            cT_ps = pps.tile([IN_DIM, M_LOC], F32, tag="pp")
            nc.tensor.transpose(cT_ps, curr_pm, identity)
            currT = consts.tile([IN_DIM, M_LOC], F32)
            nc.scalar.activation(currT, cT_ps, ACTF.Copy)

            fT_ps = pps.tile([IN_DIM, N_TAU * NT], F32, tag="pp")
            for k in range(8):
                nc.tensor.transpose(fT_ps[:, 128 * k:128 * (k + 1)],
                                    fut_pm[:, k, :], identity)
            futT = consts.tile([IN_DIM, N_TAU * NT], F32)
            nc.scalar.activation(futT, fT_ps, ACTF.Copy)

            # ---------------- node encoders (channel-major) ----------------
            s1_ps = pps.tile([H, M_LOC], F32, tag="pp")
            nc.tensor.matmul(s1_ps, ecW1_sb, currT, start=True, stop=True)
            s1 = psb.tile([H, M_LOC], F32)
            nc.scalar.activation(s1, s1_ps, ACTF.Relu, bias=ecb1_c)
            hi_ps = pps.tile([H, M_LOC], F32, tag="pp")
            nc.tensor.matmul(hi_ps, ecW2_sb, s1, start=True, stop=True)
            hiT = consts.tile([H, M_LOC], F32)
            nc.scalar.activation(hiT, hi_ps, ACTF.Relu, bias=ecb2_c)

            f1_ps = pps.tile([H, N_TAU * NT], F32, tag="pp")
            for h in range(2):
                nc.tensor.matmul(f1_ps[:, NT * h:NT * (h + 1)], efW1_sb,
                                 futT[:, NT * h:NT * (h + 1)],
                                 start=True, stop=True)
            f1 = psb.tile([H, N_TAU * NT], F32)
            nc.scalar.activation(f1, f1_ps, ACTF.Relu, bias=efb1_c)
            hj_ps = pps.tile([H, N_TAU * NT], F32, tag="pp")
            for h in range(2):
                nc.tensor.matmul(hj_ps[:, NT * h:NT * (h + 1)], efW2_sb,
                                 f1[:, NT * h:NT * (h + 1)],
                                 start=True, stop=True)
            hjT = psb.tile([H, N_TAU * NT], F32)
            nc.scalar.activation(hjT, hj_ps, ACTF.Relu, bias=efb2_c)

            # ---------------- U and V ----------------
            UT_ps = pps.tile([EH, N_TAU * M_LOC], F32, tag="pp")
            for n in range(N_TAU):
                sl = UT_ps[:, M_LOC * n:M_LOC * (n + 1)]
                nc.tensor.matmul(sl, Wi_sb, hiT, start=True, stop=False)
                nc.tensor.matmul(sl, Wxy_sb, P_rows[n],
                                 start=False, stop=True)
            UT = consts.tile([EH, N_TAU * M_LOC], F32)
            nc.scalar.activation(UT, UT_ps, ACTF.Copy)

            VT_ps = pps.tile([EH, N_TAU * NT], F32, tag="pp")
            for n in range(N_TAU):
                sl = VT_ps[:, NT * n:NT * (n + 1)]
                nc.tensor.matmul(sl, Wj_sb, hjT[:, NT * n:NT * (n + 1)],
                                 start=True, stop=False)
                nc.tensor.matmul(sl, Wxy_sb, futT[0:2, NT * n:NT * (n + 1)],
                                 start=False, stop=False)
                nc.tensor.matmul(sl, cv[n], ones_row, start=False, stop=True)
            VT_bf = consts.tile([EH, N_TAU * NT], BF16)
            nc.scalar.activation(VT_bf, VT_ps, ACTF.Copy)

        # ---------------- main edge-MLP loop ----------------
        # m-order per n: blocks j=0..31 of 4 tracks {j, 32+j, 64+j, 96+j};
        # logits accumulate into one PSUM bank partition-major (track = row)
        # via one-hot-column sign matrices, 4 column-groups concurrent.
        logits_sb = consts.tile([M_LOC, N_TAU * NT], F32)

        with tc.tile_pool(name="h1p", bufs=H1_BUFS) as h1p, \
             tc.tile_pool(name="hpp", bufs=HP_BUFS, space="PSUM") as hpp, \
             tc.tile_pool(name="h2p", bufs=H2_BUFS) as h2p, \
             tc.tile_pool(name="lgp", bufs=2, space="PSUM") as lgp:
            relu_idx = 0
            for n in range(N_TAU):
                vsl = VT_bf[:, NT * n:NT * (n + 1)]
                lg = lgp.tile([128, NT], F32, tag="lg", name=f"lg{n}")
                for j in range(32):
                    S_j = S_buf[:, 32 * j:32 * j + 32]
                    h2s = []
                    for half in range(2):      # pairs (j,32+j) and (64+j,96+j)
                        hp = hpp.tile([EH, 2 * NT], F32, tag="hp")
                        h1 = h1p.tile([EH, 2 * NT], BF16, tag="h1")
                        for i in range(2):
                            m = 32 * (2 * half + i) + j
                            ucol = UT[:, M_LOC * n + m:M_LOC * n + m + 1]
                            h1s = h1[:, NT * i:NT * (i + 1)]
                            nc.vector.tensor_scalar(
                                out=h1s, in0=vsl, scalar1=ucol, scalar2=0.0,
                                op0=ALU.add, op1=ALU.max)
                            nc.tensor.matmul(hp[:, NT * i:NT * (i + 1)],
                                             W2p_bf, h1s, start=True, stop=True)
                        h2 = h2p.tile([EH, 2 * NT], BF16, tag="h2",
                                      name=f"h2_{half}")
                        if relu_idx % DVE_H2_EVERY == DVE_H2_EVERY - 1:
                            nc.vector.tensor_scalar(
                                out=h2, in0=hp, scalar1=b2p_c, scalar2=0.0,
                                op0=ALU.add, op1=ALU.max)
                        else:
                            nc.scalar.activation(h2, hp, ACTF.Relu, bias=b2p_c)
                        relu_idx += 1
                        h2s.append(h2)
                    for q in range(4):         # 4 col groups, concurrent on PE
                        nc.tensor.matmul(
                            lg[32 * q:32 * (q + 1), :], S_j,
                            h2s[q // 2][:, NT * (q % 2):NT * (q % 2 + 1)],
                            start=(j == 0), stop=(j == 31),
                            tile_position=(0, 32 * q), skip_group_check=True)
                # logits + b3, PSUM -> SBUF, full 128 lanes
                nc.vector.tensor_scalar(
                    out=logits_sb[:, NT * n:NT * (n + 1)], in0=lg,
                    scalar1=b3_col, scalar2=None, op0=ALU.add)

        # ---------------- epilogue ----------------
        for n in range(N_TAU):
            nc.sync.dma_start(out=out.ap()[n, :, 0:NT],
                              in_=logits_sb[:, NT * n:NT * (n + 1)])
        with nc.allow_non_contiguous_dma(reason="256 single-float dummy column"):
            nc.sync.dma_start(
                out=out.ap()[:, :, NT:NT + 1],
                in_=dummy.ap().broadcast_to([N_TAU, M_LOC, 1]))

    nc.compile()
    return nc


_NC_CACHE = None


def _get_nc():
    global _NC_CACHE
    if _NC_CACHE is None:
        _NC_CACHE = build_kernel()
    return _NC_CACHE


def kernel(**inputs) -> np.ndarray:
    f = lambda x: np.ascontiguousarray(np.asarray(x), dtype=np.float32)
    curr = f(inputs["curr_nodes"])
    hist = f(inputs["hist_xy"])
    mask = f(inputs["hist_mask"])
    rep = {
        "fut_nodes": f(inputs["fut_nodes"]).reshape(N_TAU * NT, IN_DIM),
        "ec_W1": f(inputs["ec_W1"]), "ec_b1": f(inputs["ec_b1"]).reshape(1, H),
        "ec_W2": f(inputs["ec_W2"]), "ec_b2": f(inputs["ec_b2"]).reshape(1, H),
        "ef_W1": f(inputs["ef_W1"]), "ef_b1": f(inputs["ef_b1"]).reshape(1, H),
        "ef_W2": f(inputs["ef_W2"]), "ef_b2": f(inputs["ef_b2"]).reshape(1, H),
        "e_W1": f(inputs["e_W1"]), "e_b1": f(inputs["e_b1"]).reshape(1, EH),
        "e_W2": f(inputs["e_W2"]), "e_b2": f(inputs["e_b2"]).reshape(1, EH),
        "e_W3": f(inputs["e_W3"]).reshape(1, EH),
        "e_b3": f(inputs["e_b3"]).reshape(1, 1),
        "dummy_bias": f(inputs["dummy_bias"]).reshape(1, 1),
    }
    in_maps = []
    for c in range(N_CORES):
        sl = slice(c * M_LOC, (c + 1) * M_LOC)
        in_maps.append({
            "curr_nodes": curr[sl], "hist_xy": hist[sl], "hist_mask": mask[sl],
            **rep,
        })
    nc = _get_nc()
    res = run_bass_kernel_spmd(nc, in_maps, core_ids=list(range(N_CORES)))
    return np.concatenate([res.results[c]["out"] for c in range(N_CORES)],
                          axis=1)


# revision 24
# speedup vs baseline: 1.3465x; 1.3465x over previous
"""Trainium2 Bass kernel for nn_AssocModel (gnn_message_passing).

Edge-MLP logits over every (tau, track, future-node) pair:
    logits[n,m,t] = W3^T relu(W2^T relu(U[n,m] + V[n,t]) + b2) + b3
The pre-ReLU first layer decomposes exactly into an outer sum of a
per-(tau,track) vector U and a per-(tau,future) vector V, so h1 generation is
a single fused DVE tensor_scalar (broadcast-add + relu) per tile. |W3| is
folded into W2's columns (and b2) host-side, leaving a +/-1 sign vector; the
final contraction accumulates all 128 track-rows partition-major into one
PSUM bank via one-hot-column sign matrices on 4 concurrent PE column groups.

Sharding: data-parallel over the 1024 tracks -> 128 per NeuronCore, weights
and future nodes replicated; no collectives. Host packs all weight-derived
constants into two block tensors (1 DMA each) and pre-transposes the node
features; output is gathered host-side by concatenation.
"""
import os
import sys
from contextlib import ExitStack

import numpy as np
import ml_dtypes

try:
    import concourse.bass as bass
except ImportError:  # staged repo location inside the container
    sys.path.insert(0, "/opt/trn_rl_repo")
    import concourse.bass as bass

import concourse.bacc as bacc
import concourse.mybir as mybir
import concourse.tile as tile
from concourse.bass_utils import run_bass_kernel_spmd

F32 = mybir.dt.float32
BF16 = mybir.dt.bfloat16
ALU = mybir.AluOpType
ACTF = mybir.ActivationFunctionType
BFNP = ml_dtypes.bfloat16

N_CORES = 8
NM, K_HIST, IN_DIM = 1024, 10, 16
N_TAU, NT = 2, 512
H = 128      # node encoder width
EH = 128     # edge hidden width
M_LOC = NM // N_CORES  # tracks per core

# tuning knobs
ACT_COLS = 768     # h2-relu column split: ACT takes [0:ACT_COLS], DVE the rest
H1_BUFS = 6
HP_BUFS = 3        # [128,1024] f32 psum pairs in flight (2 banks each)
H2_BUFS = 3

# bf16 weight block column offsets (wb [128, WB_COLS]); every matmul lhsT
# lives at partition base 0 of its own column range.
WB_ECW1 = 0        # rows 0:16
WB_EFW1 = 128      # rows 0:16
WB_WXY = 256       # rows 0:2
WB_CV0 = 384       # row 0
WB_CV1 = 512       # row 0
WB_ECW2 = 640
WB_EFW2 = 768
WB_WI = 896
WB_WJ = 1024
WB_W2P = 1152
WB_S = 1280        # 32 matrices [128,32], column j of matrix j = sign vec
WB_COLS = WB_S + 32 * 32

# f32 weight block column offsets (wf [128, WF_COLS])
WF_IDN = 0         # identity [128,128]
WF_BIAS = 128      # bias columns: ecb1, ecb2, efb1, efb2, b2p, b3
WF_WXY = 134       # rows 0:2 = [Wx; Wy]
WF_COLS = WF_WXY + 128


def build_kernel():
    nc = bacc.Bacc(name="assoc_edge_mlp")

    def inp(name, shape, dt=F32):
        return nc.declare_dram_parameter(name, list(shape), dt, isOutput=False)

    curr = inp("curr_pm", [M_LOC, IN_DIM])
    currT = inp("currT", [IN_DIM, M_LOC], BF16)
    hist = inp("hist_xy", [M_LOC, K_HIST, 2])
    mask = inp("hist_mask", [M_LOC, K_HIST])
    futT = inp("futT", [IN_DIM, N_TAU * NT], BF16)
    wb = inp("wb", [128, WB_COLS], BF16)
    wf = inp("wf", [128, WF_COLS])
    dummy = inp("dummy_bias", [1, 1])
    out = nc.declare_dram_parameter("out", [N_TAU, M_LOC, NT + 1], F32,
                                    isOutput=True)

    _trace = bool(os.environ.get("KTRACE"))
    main_reps = int(os.environ.get("KMAINREPS", "1"))

    with tile.TileContext(nc, trace_sim=_trace) as tc, ExitStack() as stack:
        consts = stack.enter_context(tc.tile_pool(name="consts", bufs=1))

        # ---------------- inputs (7 DMAs) ----------------
        curr_pm = consts.tile([M_LOC, IN_DIM], F32)
        nc.sync.dma_start(out=curr_pm, in_=curr.ap())
        currT_sb = consts.tile([IN_DIM, M_LOC], BF16)
        nc.sync.dma_start(out=currT_sb, in_=currT.ap())
        hist_pm = consts.tile([M_LOC, K_HIST, 2], F32)
        nc.sync.dma_start(out=hist_pm, in_=hist.ap())
        mask_pm = consts.tile([M_LOC, K_HIST], F32)
        nc.sync.dma_start(out=mask_pm, in_=mask.ap())
        futT_sb = consts.tile([IN_DIM, N_TAU * NT], BF16)
        nc.sync.dma_start(out=futT_sb, in_=futT.ap())
        wb_sb = consts.tile([128, WB_COLS], BF16)
        nc.sync.dma_start(out=wb_sb, in_=wb.ap())
        wf_sb = consts.tile([128, WF_COLS], F32)
        nc.sync.dma_start(out=wf_sb, in_=wf.ap())

        ecW1_bf = wb_sb[0:IN_DIM, WB_ECW1:WB_ECW1 + H]
        efW1_bf = wb_sb[0:IN_DIM, WB_EFW1:WB_EFW1 + H]
        Wxy_bf = wb_sb[0:2, WB_WXY:WB_WXY + EH]
        cvb = [wb_sb[0:1, WB_CV0 + 128 * n:WB_CV0 + 128 * n + EH]
               for n in range(N_TAU)]
        ecW2_bf = wb_sb[:, WB_ECW2:WB_ECW2 + H]
        efW2_bf = wb_sb[:, WB_EFW2:WB_EFW2 + H]
        Wi_bf = wb_sb[:, WB_WI:WB_WI + EH]
        Wj_bf = wb_sb[:, WB_WJ:WB_WJ + EH]
        W2p_bf = wb_sb[:, WB_W2P:WB_W2P + EH]
        identity = wf_sb[:, WF_IDN:WF_IDN + 128]
        ecb1_c = wf_sb[:, WF_BIAS + 0:WF_BIAS + 1]
        ecb2_c = wf_sb[:, WF_BIAS + 1:WF_BIAS + 2]
        efb1_c = wf_sb[:, WF_BIAS + 2:WF_BIAS + 3]
        efb2_c = wf_sb[:, WF_BIAS + 3:WF_BIAS + 4]
        b2p_c = wf_sb[:, WF_BIAS + 4:WF_BIAS + 5]
        b3_c = wf_sb[:, WF_BIAS + 5:WF_BIAS + 6]
        Wxy_f32 = wf_sb[0:2, WF_WXY:WF_WXY + 128]

        ones_row = consts.tile([1, NT], BF16)
        nc.vector.memset(ones_row, 1.0)

        with tc.tile_pool(name="prep_ps", bufs=2, space="PSUM") as pps, \
             tc.tile_pool(name="prep_sb", bufs=1) as psb:
            # ---------------- velocity regression (partition-major) ----------
            t_bc = psb.tile([M_LOC, K_HIST], F32)
            nc.gpsimd.iota(t_bc, pattern=[[1, K_HIST]], base=0,
                           channel_multiplier=0,
                           allow_small_or_imprecise_dtypes=True)
            nc.vector.tensor_scalar_add(t_bc, t_bc, float(-(K_HIST - 1)))
            hx = hist_pm[:, :, 0]
            hy = hist_pm[:, :, 1]

            S0 = psb.tile([M_LOC, 1], F32)
            nc.vector.reduce_sum(S0, mask_pm, axis=mybir.AxisListType.X)
            r0 = psb.tile([M_LOC, 1], F32)
            nc.vector.tensor_scalar_max(S0, S0, 1.0)
            nc.vector.reciprocal(r0, S0)

            tmp_k = psb.tile([M_LOC, K_HIST], F32)
            t_mean = psb.tile([M_LOC, 1], F32)
            nc.vector.tensor_mul(tmp_k, mask_pm, t_bc)
            nc.vector.reduce_sum(t_mean, tmp_k, axis=mybir.AxisListType.X)
            nc.vector.tensor_mul(t_mean, t_mean, r0)

            ymx = psb.tile([M_LOC, 1], F32)
            nc.vector.tensor_mul(tmp_k, mask_pm, hx)
            nc.vector.reduce_sum(ymx, tmp_k, axis=mybir.AxisListType.X)
            nc.vector.tensor_mul(ymx, ymx, r0)
            ymy = psb.tile([M_LOC, 1], F32)
            nc.vector.tensor_mul(tmp_k, mask_pm, hy)
            nc.vector.reduce_sum(ymy, tmp_k, axis=mybir.AxisListType.X)
            nc.vector.tensor_mul(ymy, ymy, r0)

            t_c = psb.tile([M_LOC, K_HIST], F32)
            nc.vector.tensor_scalar(out=t_c, in0=t_bc, scalar1=t_mean,
                                    scalar2=None, op0=ALU.subtract)
            nc.vector.tensor_mul(t_c, t_c, mask_pm)
            y_c = psb.tile([M_LOC, K_HIST], F32)

            den = psb.tile([M_LOC, 1], F32)
            nc.vector.tensor_mul(tmp_k, t_c, t_c)
            nc.vector.reduce_sum(den, tmp_k, axis=mybir.AxisListType.X)
            nc.vector.tensor_scalar_max(den, den, 1e-8)
            rden = psb.tile([M_LOC, 1], F32)
            nc.vector.reciprocal(rden, den)

            v2 = psb.tile([M_LOC, 2], F32)
            for ci, (hc, ymc) in enumerate(((hx, ymx), (hy, ymy))):
                nc.vector.tensor_scalar(out=y_c, in0=hc, scalar1=ymc,
                                        scalar2=None, op0=ALU.subtract)
                nc.vector.tensor_mul(y_c, y_c, mask_pm)
                nc.vector.tensor_mul(tmp_k, t_c, y_c)
                num = psb.tile([M_LOC, 1], F32, name=f"num{ci}")
                nc.vector.reduce_sum(num, tmp_k, axis=mybir.AxisListType.X)
                nc.vector.tensor_mul(v2[:, ci:ci + 1], num, rden)

            # P_cols[:, 2n:2n+2] = -(xi + tau_n * v), then transpose per n
            P_cols = psb.tile([M_LOC, 4], F32)
            for n in range(N_TAU):
                sl = P_cols[:, 2 * n:2 * n + 2]
                nc.vector.tensor_scalar(out=sl, in0=v2,
                                        scalar1=float(-(n + 1)),
                                        scalar2=None, op0=ALU.mult)
                nc.vector.tensor_sub(sl, sl, curr_pm[:, 0:2])
            P_rows = []
            for n in range(N_TAU):
                PT_ps = pps.tile([2, M_LOC], F32, tag="pp", name=f"PT{n}")
                nc.tensor.transpose(PT_ps, P_cols[:, 2 * n:2 * n + 2], identity)
                Pr = psb.tile([2, M_LOC], F32, name=f"Pr{n}")
                nc.scalar.activation(Pr, PT_ps, ACTF.Copy)
                P_rows.append(Pr)

            # ---------------- node encoders (channel-major) ----------------
            s1_ps = pps.tile([H, M_LOC], F32, tag="pp")
            nc.tensor.matmul(s1_ps, ecW1_bf, currT_sb, start=True, stop=True)
            s1 = psb.tile([H, M_LOC], BF16)
            nc.scalar.activation(s1, s1_ps, ACTF.Relu, bias=ecb1_c)
            hi_ps = pps.tile([H, M_LOC], F32, tag="pp")
            nc.tensor.matmul(hi_ps, ecW2_bf, s1, start=True, stop=True)
            hiT = psb.tile([H, M_LOC], BF16)
            nc.scalar.activation(hiT, hi_ps, ACTF.Relu, bias=ecb2_c)

            f1_ps = pps.tile([H, N_TAU * NT], F32, tag="pp")
            f1 = psb.tile([H, N_TAU * NT], BF16)
            hj_ps = pps.tile([H, N_TAU * NT], F32, tag="pp")
            hjT = psb.tile([H, N_TAU * NT], BF16)
            for h in range(2):
                sl = slice(NT * h, NT * (h + 1))
                nc.tensor.matmul(f1_ps[:, sl], efW1_bf, futT_sb[:, sl],
                                 start=True, stop=True)
                nc.scalar.activation(f1[:, sl], f1_ps[:, sl], ACTF.Relu,
                                     bias=efb1_c)
                nc.tensor.matmul(hj_ps[:, sl], efW2_bf, f1[:, sl],
                                 start=True, stop=True)
                nc.scalar.activation(hjT[:, sl], hj_ps[:, sl], ACTF.Relu,
                                     bias=efb2_c)

            # ---------------- U and V ----------------
            UT_ps = pps.tile([EH, N_TAU * M_LOC], F32, tag="pp")
            UT = consts.tile([EH, N_TAU * M_LOC], F32)
            for n in range(N_TAU):
                sl = UT_ps[:, M_LOC * n:M_LOC * (n + 1)]
                nc.tensor.matmul(sl, Wi_bf, hiT, start=True, stop=False)
                nc.tensor.matmul(sl, Wxy_f32, P_rows[n], start=False, stop=True)
                nc.scalar.activation(UT[:, M_LOC * n:M_LOC * (n + 1)], sl,
                                     ACTF.Copy)

            VT_ps = pps.tile([EH, N_TAU * NT], F32, tag="pp")
            VT_bf = consts.tile([EH, N_TAU * NT], BF16)
            for n in range(N_TAU):
                sl = VT_ps[:, NT * n:NT * (n + 1)]
                nc.tensor.matmul(sl, Wj_bf, hjT[:, NT * n:NT * (n + 1)],
                                 start=True, stop=False)
                nc.tensor.matmul(sl, Wxy_bf, futT_sb[0:2, NT * n:NT * (n + 1)],
                                 start=False, stop=False)
                nc.tensor.matmul(sl, cvb[n], ones_row, start=False, stop=True)
                nc.scalar.activation(VT_bf[:, NT * n:NT * (n + 1)], sl,
                                     ACTF.Copy)

        # ---------------- main edge-MLP loop ----------------
        # m-order per n: blocks j=0..31 of 4 tracks {j, 32+j, 64+j, 96+j};
        # logits accumulate into one PSUM bank partition-major (track = row)
        # via one-hot-column sign matrices, 4 column-groups concurrent.
        logits_sb = consts.tile([M_LOC, N_TAU * NT], F32)

        with tc.tile_pool(name="h1p", bufs=H1_BUFS) as h1p, \
             tc.tile_pool(name="hpp", bufs=HP_BUFS, space="PSUM") as hpp, \
             tc.tile_pool(name="h2p", bufs=H2_BUFS) as h2p, \
             tc.tile_pool(name="lgp", bufs=2, space="PSUM") as lgp:
          for _rep in range(main_reps):
            for n in range(N_TAU):
                vsl = VT_bf[:, NT * n:NT * (n + 1)]
                lg = lgp.tile([128, NT], F32, tag="lg", name=f"lg{n}")
                for j in range(32):
                    S_j = wb_sb[:, WB_S + 32 * j:WB_S + 32 * j + 32]
                    h2s = []
                    for half in range(2):      # pairs (j,32+j) and (64+j,96+j)
                        hp = hpp.tile([EH, 2 * NT], F32, tag="hp")
                        h1 = h1p.tile([EH, 2 * NT], BF16, tag="h1")
                        for i in range(2):
                            m = 32 * (2 * half + i) + j
                            ucol = UT[:, M_LOC * n + m:M_LOC * n + m + 1]
                            h1s = h1[:, NT * i:NT * (i + 1)]
                            nc.vector.tensor_scalar(
                                out=h1s, in0=vsl, scalar1=ucol, scalar2=0.0,
                                op0=ALU.add, op1=ALU.max)
                            nc.tensor.matmul(hp[:, NT * i:NT * (i + 1)],
                                             W2p_bf, h1s, start=True, stop=True)
                        h2 = h2p.tile([EH, 2 * NT], BF16, tag="h2",
                                      name=f"h2_{half}")
                        nc.scalar.activation(h2[:, :ACT_COLS], hp[:, :ACT_COLS],
                                             ACTF.Relu, bias=b2p_c)
                        nc.vector.tensor_scalar(
                            out=h2[:, ACT_COLS:], in0=hp[:, ACT_COLS:],
                            scalar1=b2p_c, scalar2=0.0,
                            op0=ALU.add, op1=ALU.max)
                        h2s.append(h2)
                    for q in range(4):         # 4 col groups, concurrent on PE
                        nc.tensor.matmul(
                            lg[32 * q:32 * (q + 1), :], S_j,
                            h2s[q // 2][:, NT * (q % 2):NT * (q % 2 + 1)],
                            start=(j == 0), stop=(j == 31),
                            tile_position=(0, 32 * q), skip_group_check=True)
                # logits + b3, PSUM -> SBUF, full 128 lanes
                nc.vector.tensor_scalar(
                    out=logits_sb[:, NT * n:NT * (n + 1)], in0=lg,
                    scalar1=b3_c, scalar2=None, op0=ALU.add)

        # ---------------- output ----------------
        for n in range(N_TAU):
            nc.sync.dma_start(out=out.ap()[n, :, 0:NT],
                              in_=logits_sb[:, NT * n:NT * (n + 1)])
        with nc.allow_non_contiguous_dma(reason="256 single-float dummy column"):
            nc.sync.dma_start(
                out=out.ap()[:, :, NT:NT + 1],
                in_=dummy.ap().broadcast_to([N_TAU, M_LOC, 1]))

    nc.compile()
    return nc


def pack_weights(inputs):
    """Host-side weight preprocessing into the wb (bf16) / wf (f32) blocks."""
    f = lambda k: np.asarray(inputs[k], dtype=np.float32)
    e_W1, e_b1 = f("e_W1"), f("e_b1").reshape(EH)
    e_W2, e_b2 = f("e_W2"), f("e_b2").reshape(EH)
    w3, b3 = f("e_W3").reshape(EH), f("e_b3").reshape(1)

    s = np.where(w3 >= 0.0, 1.0, -1.0).astype(np.float32)
    w3a = np.abs(w3)
    W2p = (e_W2 * w3a[None, :]).astype(np.float32)
    b2p = (e_b2 * w3a).astype(np.float32)

    wb = np.zeros((128, WB_COLS), dtype=BFNP)
    wb[0:IN_DIM, WB_ECW1:WB_ECW1 + H] = f("ec_W1")
    wb[0:IN_DIM, WB_EFW1:WB_EFW1 + H] = f("ef_W1")
    wb[0:2, WB_WXY:WB_WXY + EH] = e_W1[2 * H:2 * H + 2]   # [Wx; Wy]
    for n in range(N_TAU):                           # cv_n = b1 + tau_n*Wt
        wb[0, WB_CV0 + 128 * n:WB_CV0 + 128 * n + EH] = (
            e_b1 + (n + 1) / N_TAU * e_W1[2 * H + 2])
    wb[:, WB_ECW2:WB_ECW2 + H] = f("ec_W2")
    wb[:, WB_EFW2:WB_EFW2 + H] = f("ef_W2")
    wb[:, WB_WI:WB_WI + EH] = e_W1[0:H]
    wb[:, WB_WJ:WB_WJ + EH] = e_W1[H:2 * H]
    wb[:, WB_W2P:WB_W2P + EH] = W2p
    for j in range(32):                              # sign one-hot matrices
        wb[:, WB_S + 32 * j + j] = s

    wf = np.zeros((128, WF_COLS), dtype=np.float32)
    wf[:, WF_IDN:WF_IDN + 128] = np.eye(128, dtype=np.float32)
    for i, v in enumerate((f("ec_b1").reshape(H), f("ec_b2").reshape(H),
                           f("ef_b1").reshape(H), f("ef_b2").reshape(H),
                           b2p, np.full(128, b3[0], np.float32))):
        wf[:, WF_BIAS + i] = v
    wf[0:2, WF_WXY:WF_WXY + 128] = e_W1[2 * H:2 * H + 2]
    return wb, wf


_NC_CACHE = None


def _get_nc():
    global _NC_CACHE
    if _NC_CACHE is None:
        _NC_CACHE = build_kernel()
    return _NC_CACHE


def make_in_maps(inputs):
    f32 = lambda x: np.ascontiguousarray(np.asarray(x), dtype=np.float32)
    curr = f32(inputs["curr_nodes"])
    hist = f32(inputs["hist_xy"])
    mask = f32(inputs["hist_mask"])
    fut = f32(inputs["fut_nodes"]).reshape(N_TAU * NT, IN_DIM)
    wb, wf = pack_weights(inputs)
    rep = {
        "futT": np.ascontiguousarray(fut.T.astype(BFNP)),
        "wb": wb, "wf": wf,
        "dummy_bias": f32(inputs["dummy_bias"]).reshape(1, 1),
    }
    in_maps = []
    for c in range(N_CORES):
        sl = slice(c * M_LOC, (c + 1) * M_LOC)
        in_maps.append({
            "curr_pm": curr[sl],
            "currT": np.ascontiguousarray(curr[sl].T.astype(BFNP)),
            "hist_xy": hist[sl], "hist_mask": mask[sl],
            **rep,
        })
    return in_maps


def kernel(**inputs) -> np.ndarray:
    nc = _get_nc()
    in_maps = make_in_maps(inputs)
    res = run_bass_kernel_spmd(nc, in_maps, core_ids=list(range(N_CORES)))
    return np.concatenate([res.results[c]["out"] for c in range(N_CORES)],
                          axis=1)
